# revision 1
# baseline (speedup 1.0000x reference)
"""Gumbel Top-K gate kernel for Trainium2 (8 NeuronCores, SPMD).

Math: mask[b, 0, r, m] = 1 iff z[b, r, m] is among the top-16 of row r, where
  z = mean_h(q_h k_h^T)/sqrt(64) + gumbel(u),  gumbel = -log(-log(u+eps)+eps).
Softmax is strictly monotone per row, so the reference's softmax/top-k mask
equals thresholding z at its 16th-largest value per row (ties included via >=).

Sharding: core c handles batch b = c//2, row half c%2 -> [1024, 2048] slab.
Head-mean folds into one [1024, 512] x [512, 2048] matmul per core (concat
heads along the contraction dim). Host prep hands each core d-major (already
transposed) qT [512, 1024] (pre-scaled by the exact power-of-two 1/64 =
1/sqrt(64) * 1/8 head-mean) and kT [512, 2048], so the PE does zero
transposes.

Engine split per 128-row tile: PE: 16 accumulating fp32 score matmuls;
ACT: two Ln passes for the gumbel; DVE: z = S - g2 (PSUM read), the top-16
threshold (max8 -> match_replace -> max8), and the >= compare writing a
uint8 mask (exact 0/1; widened to f32 on host).
"""

import sys

sys.path.insert(0, "/opt/trn_rl_repo")

import numpy as np

import concourse.bacc as bacc
import concourse.mybir as mybir
import concourse.tile as tile
from concourse import bass_utils

B, H, N, D = 4, 8, 2048, 64
HD = H * D  # 512 contraction dim (heads concatenated)
N_CORES = 8
ROWS = N * B // N_CORES  # 1024 rows per core
P = 128
EPS = 1e-9
NEG_BIG = -3.0e38
F32 = mybir.dt.float32
F32R = mybir.dt.float32r
U8 = mybir.dt.uint8


def _make_identity(nc, ident, fill):
    nc.gpsimd.memset(ident, 0.0)
    sq = ident.shape[0]
    nc.gpsimd.affine_select(
        out=ident,
        in_=ident,
        compare_op=mybir.AluOpType.not_equal,
        fill=fill,
        base=0,
        pattern=[[-1, sq]],
        channel_multiplier=1,
    )


def _build_body(tc, qT_d, kT_d, u_d, mask_d):
    nc = tc.nc
    n_rtiles = ROWS // P  # 8
    n_c = HD // P  # 4 contraction chunks
    act = mybir.ActivationFunctionType

    with (
        tc.tile_pool(name="consts", bufs=1) as consts,
        tc.tile_pool(name="kqT", bufs=1) as kqT_pool,
        tc.tile_pool(name="s_psum", bufs=2, space="PSUM") as s_psum,
        tc.tile_pool(name="work", bufs=2) as work,
        tc.tile_pool(name="uin", bufs=3) as uin,
        tc.tile_pool(name="mout", bufs=2) as mout,
        tc.tile_pool(name="small", bufs=2) as small,
    ):
        eps_tile = consts.tile([P, 1], F32)
        nc.vector.memset(eps_tile, EPS)

        u_t = u_d.rearrange("(t p) n -> t p n", p=P)
        mask_t = mask_d.rearrange("(t p) n -> t p n", p=P)
        # prefetch tile 0's noise ahead of the weight loads so ACT starts early
        ut0 = uin.tile([P, N], F32, tag="u")
        nc.sync.dma_start(out=ut0, in_=u_t[0])

        # d-major loads straight from host-transposed DRAM; no PE transposes.
        # One DMA per 128-d chunk so c=0 matmuls start after 1 MiB, not 6 MiB.
        kT_r = kT_d.rearrange("(c p) m -> c p m", p=P)
        qT_r = qT_d.rearrange("(c p) m -> c p m", p=P)
        kT = [kqT_pool.tile([P, N], F32, tag=f"kT{c}", name=f"kT{c}") for c in range(n_c)]
        qT = [kqT_pool.tile([P, ROWS], F32, tag=f"qT{c}", name=f"qT{c}") for c in range(n_c)]
        for c in range(n_c):
            nc.sync.dma_start(out=kT[c], in_=kT_r[c])
            nc.sync.dma_start(out=qT[c], in_=qT_r[c])

        for t in range(n_rtiles):
            if t == 0:
                ut = ut0
            else:
                ut = uin.tile([P, N], F32, tag="u")
                nc.sync.dma_start(out=ut, in_=u_t[t])
            g1 = work.tile([P, N], F32, tag="g1")
            nc.scalar.activation(g1, ut, act.Ln, bias=eps_tile, scale=1.0)
            # g2 = log(-log(u+eps)+eps); z = S - g2
            g2 = work.tile([P, N], F32, tag="g2")
            nc.scalar.activation(g2, g1, act.Ln, bias=eps_tile, scale=-1.0)

            S = s_psum.tile([P, N], F32, tag="S")  # 4 PSUM banks
            for c in range(n_c):
                for m in range(4):
                    nc.tensor.matmul(
                        S[:, m * 512 : (m + 1) * 512],
                        qT[c][:, t * P : (t + 1) * P],
                        kT[c][:, m * 512 : (m + 1) * 512],
                        start=(c == 0),
                        stop=(c == n_c - 1),
                    )

            z = work.tile([P, N], F32, tag="z")
            nc.vector.tensor_sub(z, S, g2)  # PSUM read + gumbel add on DVE

            m8a = small.tile([P, 8], F32, tag="m8a")
            nc.vector.max(out=m8a, in_=z)
            zs = work.tile([P, N], F32, tag="zs")
            nc.vector.match_replace(
                out=zs, in_to_replace=m8a, in_values=z, imm_value=NEG_BIG
            )
            m8b = small.tile([P, 8], F32, tag="m8b")
            nc.vector.max(out=m8b, in_=zs)

            mk = mout.tile([P, N], U8, tag="mk")
            nc.vector.tensor_scalar(
                out=mk,
                in0=z,
                scalar1=m8b[:, 7:8],
                scalar2=None,
                op0=mybir.AluOpType.is_ge,
            )
            nc.sync.dma_start(out=mask_t[t], in_=mk)


def build_kernel():
    nc = bacc.Bacc(
        "TRN2", target_bir_lowering=False, debug=False, num_devices=N_CORES
    )
    qT = nc.dram_tensor("qT", [HD, ROWS], F32, kind="ExternalInput").ap()
    kT = nc.dram_tensor("kT", [HD, N], F32, kind="ExternalInput").ap()
    u = nc.dram_tensor("u", [ROWS, N], F32, kind="ExternalInput").ap()
    mask = nc.dram_tensor("mask", [ROWS, N], U8, kind="ExternalOutput").ap()
    with tile.TileContext(nc) as tc:
        _build_body(tc, qT, kT, u, mask)
    nc.compile()
    return nc


_NC_CACHE = None
LAST_RESULTS = None


def _get_nc():
    global _NC_CACHE
    if _NC_CACHE is None:
        _NC_CACHE = build_kernel()
    return _NC_CACHE


def make_in_maps(q, k, u):
    q = np.asarray(q, np.float32)
    k = np.asarray(k, np.float32)
    u = np.asarray(u, np.float32)
    in_maps = []
    kT_by_batch = {}
    for core in range(N_CORES):
        b, half = divmod(core, 2)
        r0 = half * ROWS
        if b not in kT_by_batch:
            # [N, H, D] -> [H*D, N] d-major
            kT_by_batch[b] = np.ascontiguousarray(
                k[b].transpose(1, 0, 2).reshape(N, HD).T
            )
        # 1/64 scale is an exact power-of-two: bit-identical to on-chip scaling
        qT = np.ascontiguousarray(
            q[b, :, r0 : r0 + ROWS, :].transpose(1, 0, 2).reshape(ROWS, HD).T
            * np.float32(1.0 / 64)
        )
        in_maps.append(
            {
                "qT": qT,
                "kT": kT_by_batch[b],
                "u": np.ascontiguousarray(u[b, r0 : r0 + ROWS]),
            }
        )
    return in_maps


def kernel(q, k, u):
    global LAST_RESULTS
    in_maps = make_in_maps(q, k, u)
    res = bass_utils.run_bass_kernel_spmd(
        _get_nc(), in_maps, core_ids=list(range(N_CORES))
    )
    LAST_RESULTS = res
    out = np.empty((B, 1, N, N), np.float32)
    for core in range(N_CORES):
        b, half = divmod(core, 2)
        r0 = half * ROWS
        out[b, 0, r0 : r0 + ROWS] = res.results[core]["mask"].astype(np.float32)
    return out



# revision 12
# speedup vs baseline: 1.5175x; 1.5175x over previous
"""Gumbel Top-K gate kernel for Trainium2 (8 NeuronCores, SPMD).

Math: mask[b, 0, r, m] = 1 iff z[b, r, m] is among the top-16 of row r, where
  z = mean_h(q_h k_h^T)/sqrt(64) + gumbel(u),  gumbel = -log(-log(u+eps)+eps).
Softmax is strictly monotone per row, so the reference's softmax/top-k mask
equals thresholding z at its 16th-largest value per row (ties included via >=).

Exponential-race reformulation: with g1 = log(u+eps), top-16 of z equals
top-16 of y = g1 * exp(-S) (strictly monotone map y = -exp(-z); the
reference's outer eps only shifts elements deep inside the top-16, never
boundary elements, so it is dropped). ACT evacuates PSUM (Exp, scale=-1/8
folds the head-mean), GpSimd computes y (SBUF-only tensor_tensor), and the
DVE keeps only the top-k search plus the final compare.

Top-16 threshold via segmented max8: 8x max8 over 256-col segments -> 64
candidates; the 16th-largest candidate equals the true row threshold unless
one segment holds >8 of the row's top-16 (P ~ 4e-5 per row). Measured flips
vs the fp32 reference on the actual inputs: 7 of 16.7M (rel err 7.3e-3,
budget 2e-2).

Matmul in fp16 (q pre-scaled by 1/8 on host, exact power of two): 1
cycle/row on the PE vs fp32's 4.

Tiles are processed in PAIRS ([128, 4096] working set, two 128-row tiles
side by side in the free dim): 4 loop iterations instead of 8 halves the
per-op fixed costs (GpSimd ~1.4us/op dispatch) and the semaphore/DMA queue
traffic that otherwise dominates. Activation tables are pinned to the
combined ln+exp set to stop the per-tile table reload thrash.

Sharding: core c handles batch b = c//2, row half c%2 -> [1024, 2048] slab.
"""

import sys

sys.path.insert(0, "/opt/trn_rl_repo")

import numpy as np

import concourse.bacc as bacc
import concourse.mybir as mybir
import concourse.tile as tile
from concourse import bass_utils

B, H, N, D = 4, 8, 2048, 64
HD = H * D  # 512 contraction dim (heads concatenated)
N_CORES = 8
ROWS = N * B // N_CORES  # 1024 rows per core
P = 128
EPS = 1e-9
NEG_BIG = -3.0e38
NSEG = 8
SEG = N // NSEG  # 256
F32 = mybir.dt.float32
F16 = mybir.dt.float16
U8 = mybir.dt.uint8


def _pin_act_tables(arch):
    """Force Ln+Exp onto the combined ACT table set. The table-load pass
    picks the first set containing each function (Ln -> natural_log,
    Exp -> exp_and_others), reloading tables every tile (~1.3us each, 18us
    total). Stripping Ln/Exp from every other set makes both resolve to
    natural_log_exp_and_others; set indices are unchanged so the emitted
    act_func_set_id stays valid."""
    from concourse.hw_specs import get_activation_tables

    tabs = get_activation_tables(arch)  # functools.cache -> shared dict
    keep = "natural_log_exp_and_others"
    if keep in tabs:
        for name, funcs in tabs.items():
            if name != keep:
                funcs.discard(mybir.ActivationFunctionType.Ln)
                funcs.discard(mybir.ActivationFunctionType.Exp)


def _build_body(tc, qT_d, kT_d, u_d, mask_d):
    nc = tc.nc
    n_pairs = ROWS // (2 * P)  # 4
    n_c = HD // P  # 4 contraction chunks
    act = mybir.ActivationFunctionType
    alu = mybir.AluOpType
    N2 = 2 * N  # 4096 free elems per paired working tile

    with (
        tc.tile_pool(name="consts", bufs=1) as consts,
        tc.tile_pool(name="kqT", bufs=1) as kqT_pool,
        tc.tile_pool(name="s_psum", bufs=1, space="PSUM") as s_psum,
        tc.tile_pool(name="work", bufs=2) as work,
        tc.tile_pool(name="uin", bufs=2) as uin,
        tc.tile_pool(name="mout", bufs=2) as mout,
        tc.tile_pool(name="small", bufs=2) as small,
    ):
        eps_tile = consts.tile([P, 1], F32)
        nc.vector.memset(eps_tile, EPS)

        # pair pr, half h, partition p, col n  <->  u row = pr*256 + h*128 + p
        u_t = u_d.rearrange("(t p) n -> t p n", p=P)
        mask_t = mask_d.rearrange("(t p) n -> t p n", p=P)
        # prefetch pair 0's noise ahead of the weight loads so ACT starts early
        ut0 = uin.tile([P, N2], F32, tag="u")
        nc.sync.dma_start(out=ut0[:, :N], in_=u_t[0])
        nc.sync.dma_start(out=ut0[:, N:], in_=u_t[1])

        # d-major fp16 loads straight from host-transposed DRAM; no PE transposes.
        kT_r = kT_d.rearrange("(c p) m -> c p m", p=P)
        qT_r = qT_d.rearrange("(c p) m -> c p m", p=P)
        kT = [kqT_pool.tile([P, N], F16, tag=f"kT{c}", name=f"kT{c}") for c in range(n_c)]
        qT = [kqT_pool.tile([P, ROWS], F16, tag=f"qT{c}", name=f"qT{c}") for c in range(n_c)]
        for c in range(n_c):
            nc.sync.dma_start(out=kT[c], in_=kT_r[c])
            nc.sync.dma_start(out=qT[c], in_=qT_r[c])

        for pr in range(n_pairs):
            if pr == 0:
                ut = ut0
            else:
                ut = uin.tile([P, N2], F32, tag="u")
                nc.sync.dma_start(out=ut[:, :N], in_=u_t[2 * pr])
                nc.sync.dma_start(out=ut[:, N:], in_=u_t[2 * pr + 1])
            g1 = work.tile([P, N2], F32, tag="g1")
            nc.scalar.activation(g1, ut, act.Ln, bias=eps_tile, scale=1.0)

            # two PSUM tiles per pair (4 banks each); matmuls per 128-row half
            e = work.tile([P, N2], F32, tag="e")
            for h in range(2):
                t = 2 * pr + h
                S = s_psum.tile([P, N], F32, tag=f"S{h}")
                for c in range(n_c):
                    for m in range(4):
                        nc.tensor.matmul(
                            S[:, m * 512 : (m + 1) * 512],
                            qT[c][:, t * P : (t + 1) * P],
                            kT[c][:, m * 512 : (m + 1) * 512],
                            start=(c == 0),
                            stop=(c == n_c - 1),
                        )
                # e = exp(-S/8) evacuates PSUM on ACT (head-mean folded in)
                nc.scalar.activation(
                    e[:, h * N : (h + 1) * N], S, act.Exp, scale=-0.125
                )

            # y = g1 * e  ~= -exp(-z): one SBUF-only GpSimd op per pair
            y = work.tile([P, N2], F32, tag="y")
            nc.gpsimd.tensor_tensor(y, g1, e, op=alu.mult)

            # per half: top-8 of each 256-col segment -> 64 candidates, then
            # 16th largest of candidates = threshold. One merged small tile:
            # half h at base 144h: [0:64) cand | [64:72) c8a | [72:136) cand2
            # | [136:144) c8b
            sm = small.tile([P, 288], F32, tag="sm")
            mk = mout.tile([P, N2], U8, tag="mk")
            for h in range(2):
                b0 = 144 * h
                y_h = y[:, h * N : (h + 1) * N]
                for s in range(NSEG):
                    nc.vector.max(
                        out=sm[:, b0 + 8 * s : b0 + 8 * (s + 1)],
                        in_=y_h[:, SEG * s : SEG * (s + 1)],
                    )
                nc.vector.max(out=sm[:, b0 + 64 : b0 + 72], in_=sm[:, b0 : b0 + 64])
                nc.vector.match_replace(
                    out=sm[:, b0 + 72 : b0 + 136],
                    in_to_replace=sm[:, b0 + 64 : b0 + 72],
                    in_values=sm[:, b0 : b0 + 64],
                    imm_value=NEG_BIG,
                )
                nc.vector.max(
                    out=sm[:, b0 + 136 : b0 + 144], in_=sm[:, b0 + 72 : b0 + 136]
                )
                nc.vector.tensor_scalar(
                    out=mk[:, h * N : (h + 1) * N],
                    in0=y_h,
                    scalar1=sm[:, b0 + 143 : b0 + 144],
                    scalar2=None,
                    op0=alu.is_ge,
                )
            nc.sync.dma_start(out=mask_t[2 * pr], in_=mk[:, :N])
            nc.sync.dma_start(out=mask_t[2 * pr + 1], in_=mk[:, N:])


def build_kernel():
    nc = bacc.Bacc(
        "TRN2", target_bir_lowering=False, debug=False, num_devices=N_CORES
    )
    _pin_act_tables(nc.m.arch)
    qT = nc.dram_tensor("qT", [HD, ROWS], F16, kind="ExternalInput").ap()
    kT = nc.dram_tensor("kT", [HD, N], F16, kind="ExternalInput").ap()
    u = nc.dram_tensor("u", [ROWS, N], F32, kind="ExternalInput").ap()
    mask = nc.dram_tensor("mask", [ROWS, N], U8, kind="ExternalOutput").ap()
    with tile.TileContext(nc) as tc:
        _build_body(tc, qT, kT, u, mask)
    nc.compile()
    return nc


_NC_CACHE = None
LAST_RESULTS = None


def _get_nc():
    global _NC_CACHE
    if _NC_CACHE is None:
        _NC_CACHE = build_kernel()
    return _NC_CACHE


def make_in_maps(q, k, u):
    q = np.asarray(q, np.float32)
    k = np.asarray(k, np.float32)
    u = np.asarray(u, np.float32)
    in_maps = []
    kT_by_batch = {}
    for core in range(N_CORES):
        b, half = divmod(core, 2)
        r0 = half * ROWS
        if b not in kT_by_batch:
            # [N, H, D] -> [H*D, N] d-major, fp16
            kT_by_batch[b] = np.ascontiguousarray(
                k[b].transpose(1, 0, 2).reshape(N, HD).T
            ).astype(np.float16)
        # 1/8 = 1/sqrt(64) is an exact power of two: no extra rounding before
        # the fp16 cast; the head-mean 1/8 is folded into Exp's scale on-chip
        qT = np.ascontiguousarray(
            q[b, :, r0 : r0 + ROWS, :].transpose(1, 0, 2).reshape(ROWS, HD).T
            * np.float32(1.0 / 8)
        ).astype(np.float16)
        in_maps.append(
            {
                "qT": qT,
                "kT": kT_by_batch[b],
                "u": np.ascontiguousarray(u[b, r0 : r0 + ROWS]),
            }
        )
    return in_maps


def kernel(q, k, u):
    global LAST_RESULTS
    in_maps = make_in_maps(q, k, u)
    res = bass_utils.run_bass_kernel_spmd(
        _get_nc(), in_maps, core_ids=list(range(N_CORES))
    )
    LAST_RESULTS = res
    out = np.empty((B, 1, N, N), np.float32)
    for core in range(N_CORES):
        b, half = divmod(core, 2)
        r0 = half * ROWS
        out[b, 0, r0 : r0 + ROWS] = res.results[core]["mask"].astype(np.float32)
    return out


# revision 14
# speedup vs baseline: 1.8659x; 1.2296x over previous
"""Gumbel Top-K gate kernel for Trainium2 (8 NeuronCores, SPMD).

Math: mask[b, 0, r, m] = 1 iff z[b, r, m] is among the top-16 of row r, where
  z = mean_h(q_h k_h^T)/sqrt(64) + gumbel(u),  gumbel = -log(-log(u+eps)+eps).
Softmax is strictly monotone per row, so the reference's softmax/top-k mask
equals thresholding z at its 16th-largest value per row (ties included via >=).

Engine split (the DVE and GpSimd share an SBUF port, so GpSimd offload is
counterproductive — GpSimd is unused; ACT has its own ports):
  ACT: g1 = Ln(u+eps); g2 = Ln(-g1+eps); final compare as
       mask_u8 = Sign(z - t16m) (Sign clamps {-1,0,1} -> {0,1} on the u8
       output path; t16m is 1-2 ulp below t16 so z == t16 lands at 1).
  PE:  16 fp16 matmuls per 128-row tile (q pre-scaled by 1/8 on host; PSUM
       holds 8*logits).
  DVE: one fused scalar_tensor_tensor z = (PSUM*0.125) - g2 (PSUM
       evacuation + head-mean + gumbel combine in one 1x pass), then the
       top-k search: 8x max8 over 256-col segments -> 64 candidates, then
       max8/match_replace/max8 on the candidates -> t16 = 16th largest.

Segmented max8 is exact unless one segment holds >8 of the row's top-16
(P ~ 4e-5 per row). Measured flips vs the fp32 reference on the actual
inputs: 7 of 16.7M (rel err 7.3e-3, budget 2e-2) — from the fp16 matmul
(6) and the segment assumption (1).

The mask stage for tile t is emitted after tile t+1's Ln passes (manual
software pipelining) so the in-order ACT queue never stalls waiting for the
DVE chain. Activation tables are pinned to one set to avoid reload thrash.

Sharding: core c handles batch b = c//2, row half c%2 -> [1024, 2048] slab.
"""

import sys

sys.path.insert(0, "/opt/trn_rl_repo")

import numpy as np

import concourse.bacc as bacc
import concourse.mybir as mybir
import concourse.tile as tile
from concourse import bass_utils

B, H, N, D = 4, 8, 2048, 64
HD = H * D  # 512 contraction dim (heads concatenated)
N_CORES = 8
ROWS = N * B // N_CORES  # 1024 rows per core
P = 128
EPS = 1e-9
NEG_BIG = -3.0e38
NSEG = 8
SEG = N // NSEG  # 256
# t16m = t16 * (1 - 2^-22): 1-2 ulp below t16 (t16 > 0 w.o.p.), so
# Sign(z - t16m) is +1 exactly when z >= t16.
T16_SHRINK = -(1.0 - 2.0**-22)  # negated: used directly as the Sign bias
F32 = mybir.dt.float32
F16 = mybir.dt.float16
U8 = mybir.dt.uint8


def _pin_act_tables(arch):
    """Pin Ln (and Exp, if ever used) to the combined table set so the ACT
    engine never reloads tables mid-kernel; Sign is present in every set."""
    from concourse.hw_specs import get_activation_tables

    tabs = get_activation_tables(arch)  # functools.cache -> shared dict
    keep = "natural_log_exp_and_others"
    if keep in tabs:
        for name, funcs in tabs.items():
            if name != keep:
                funcs.discard(mybir.ActivationFunctionType.Ln)
                funcs.discard(mybir.ActivationFunctionType.Exp)


def _build_body(tc, qT_d, kT_d, u_d, mask_d):
    nc = tc.nc
    n_rtiles = ROWS // P  # 8
    n_c = HD // P  # 4 contraction chunks
    act = mybir.ActivationFunctionType
    alu = mybir.AluOpType

    with (
        tc.tile_pool(name="consts", bufs=1) as consts,
        tc.tile_pool(name="kqT", bufs=1) as kqT_pool,
        tc.tile_pool(name="s_psum", bufs=2, space="PSUM") as s_psum,
        tc.tile_pool(name="work", bufs=3) as work,
        tc.tile_pool(name="uin", bufs=3) as uin,
        tc.tile_pool(name="mout", bufs=2) as mout,
        tc.tile_pool(name="small", bufs=2) as small,
    ):
        eps_tile = consts.tile([P, 1], F32)
        nc.vector.memset(eps_tile, EPS)

        u_t = u_d.rearrange("(t p) n -> t p n", p=P)
        mask_t = mask_d.rearrange("(t p) n -> t p n", p=P)
        # prefetch tile 0's noise ahead of the weight loads so ACT starts early
        ut0 = uin.tile([P, N], F32, tag="u")
        nc.sync.dma_start(out=ut0, in_=u_t[0])

        # d-major fp16 loads straight from host-transposed DRAM; no PE transposes.
        kT_r = kT_d.rearrange("(c p) m -> c p m", p=P)
        qT_r = qT_d.rearrange("(c p) m -> c p m", p=P)
        kT = [kqT_pool.tile([P, N], F16, tag=f"kT{c}", name=f"kT{c}") for c in range(n_c)]
        qT = [kqT_pool.tile([P, ROWS], F16, tag=f"qT{c}", name=f"qT{c}") for c in range(n_c)]
        for c in range(n_c):
            nc.sync.dma_start(out=kT[c], in_=kT_r[c])
            nc.sync.dma_start(out=qT[c], in_=qT_r[c])

        pending = None  # (z, tb, t) of the previous tile, mask stage not yet emitted

        def emit_mask(zp, smp, tp):
            mk = mout.tile([P, N], U8, tag="mk")
            nc.scalar.activation(mk, zp, act.Sign, bias=smp[:, 144:145], scale=1.0)
            nc.sync.dma_start(out=mask_t[tp], in_=mk)

        for t in range(n_rtiles):
            if t == 0:
                ut = ut0
            else:
                ut = uin.tile([P, N], F32, tag="u")
                nc.sync.dma_start(out=ut, in_=u_t[t])
            g1 = work.tile([P, N], F32, tag="g1")
            nc.scalar.activation(g1, ut, act.Ln, bias=eps_tile, scale=1.0)
            g2 = work.tile([P, N], F32, tag="g2")
            nc.scalar.activation(g2, g1, act.Ln, bias=eps_tile, scale=-1.0)

            M = s_psum.tile([P, N], F32, tag="M")  # 4 PSUM banks, holds 8*logits
            for c in range(n_c):
                for m in range(4):
                    nc.tensor.matmul(
                        M[:, m * 512 : (m + 1) * 512],
                        qT[c][:, t * P : (t + 1) * P],
                        kT[c][:, m * 512 : (m + 1) * 512],
                        start=(c == 0),
                        stop=(c == n_c - 1),
                    )

            # z = M/8 - g2: PSUM evacuation + head-mean + gumbel in one DVE op
            z = work.tile([P, N], F32, tag="z")
            nc.vector.scalar_tensor_tensor(
                z, M, 0.125, g2, alu.mult, alu.subtract
            )

            # top-8 of each 256-col segment -> 64 candidates; 16th largest of
            # candidates = row threshold. Merged small tile:
            # [0:64) cand | [64:72) c8a | [72:136) cand2 | [136:144) c8b
            # [144:145) tb = -t16m (Sign bias)
            sm = small.tile([P, 145], F32, tag="sm")
            for s in range(NSEG):
                nc.vector.max(out=sm[:, 8 * s : 8 * (s + 1)], in_=z[:, SEG * s : SEG * (s + 1)])
            nc.vector.max(out=sm[:, 64:72], in_=sm[:, 0:64])
            nc.vector.match_replace(
                out=sm[:, 72:136], in_to_replace=sm[:, 64:72],
                in_values=sm[:, 0:64], imm_value=NEG_BIG,
            )
            nc.vector.max(out=sm[:, 136:144], in_=sm[:, 72:136])
            nc.vector.tensor_scalar(
                out=sm[:, 144:145], in0=sm[:, 143:144],
                scalar1=T16_SHRINK, scalar2=None, op0=alu.mult,
            )

            if pending is not None:
                emit_mask(*pending)
            pending = (z, sm, t)

        emit_mask(*pending)


def build_kernel():
    nc = bacc.Bacc(
        "TRN2", target_bir_lowering=False, debug=False, num_devices=N_CORES
    )
    _pin_act_tables(nc.m.arch)
    qT = nc.dram_tensor("qT", [HD, ROWS], F16, kind="ExternalInput").ap()
    kT = nc.dram_tensor("kT", [HD, N], F16, kind="ExternalInput").ap()
    u = nc.dram_tensor("u", [ROWS, N], F32, kind="ExternalInput").ap()
    mask = nc.dram_tensor("mask", [ROWS, N], U8, kind="ExternalOutput").ap()
    with tile.TileContext(nc) as tc:
        _build_body(tc, qT, kT, u, mask)
    nc.compile()
    return nc


_NC_CACHE = None
LAST_RESULTS = None


def _get_nc():
    global _NC_CACHE
    if _NC_CACHE is None:
        _NC_CACHE = build_kernel()
    return _NC_CACHE


def make_in_maps(q, k, u):
    q = np.asarray(q, np.float32)
    k = np.asarray(k, np.float32)
    u = np.asarray(u, np.float32)
    in_maps = []
    kT_by_batch = {}
    for core in range(N_CORES):
        b, half = divmod(core, 2)
        r0 = half * ROWS
        if b not in kT_by_batch:
            # [N, H, D] -> [H*D, N] d-major, fp16
            kT_by_batch[b] = np.ascontiguousarray(
                k[b].transpose(1, 0, 2).reshape(N, HD).T
            ).astype(np.float16)
        # 1/8 = 1/sqrt(64) is an exact power of two: no extra rounding before
        # the fp16 cast; the head-mean 1/8 is applied in the z combine on-chip
        qT = np.ascontiguousarray(
            q[b, :, r0 : r0 + ROWS, :].transpose(1, 0, 2).reshape(ROWS, HD).T
            * np.float32(1.0 / 8)
        ).astype(np.float16)
        in_maps.append(
            {
                "qT": qT,
                "kT": kT_by_batch[b],
                "u": np.ascontiguousarray(u[b, r0 : r0 + ROWS]),
            }
        )
    return in_maps


def kernel(q, k, u):
    global LAST_RESULTS
    in_maps = make_in_maps(q, k, u)
    res = bass_utils.run_bass_kernel_spmd(
        _get_nc(), in_maps, core_ids=list(range(N_CORES))
    )
    LAST_RESULTS = res
    out = np.empty((B, 1, N, N), np.float32)
    for core in range(N_CORES):
        b, half = divmod(core, 2)
        r0 = half * ROWS
        out[b, 0, r0 : r0 + ROWS] = res.results[core]["mask"].astype(np.float32)
    return out
